# revision 1
# baseline (speedup 1.0000x reference)
"""Windowed attention (swin-style, 49-token windows, 8 heads) with DynamicPosBias.

Strategy: data-parallel over B=2048 windows -> 256 windows/core on 8 cores.
Host pre-transposes q,k per head to [W, 8, 64, 49] so the device needs no
transposes; v gets a fused ones-column so PV matmul also produces the softmax
denominator. Position-bias MLP runs on device once; the bias is fused into the
QK matmul as 49 extra contraction rows (lhsT=[K^T;I49], rhs=[Q^T;8*rpb]) and
exp(0.125*S) folds the 1/sqrt(64) scale.
"""

import numpy as np
from contextlib import ExitStack

import concourse.bass as bass
import concourse.mybir as mybir
import concourse.tile as tile
from concourse import bacc
from concourse.bass_utils import run_bass_kernel_spmd

G = 7
NTOK = 49          # tokens per window
H = 8              # heads
HD = 64            # head dim
C = 512
B = 2048
NCORES = 8
W = B // NCORES    # windows per core
T = (2 * G - 1) ** 2  # 169 bias table entries
PDIM = 32          # MLP hidden
NBUF = 4
F32 = mybir.dt.float32
F16 = mybir.dt.float16
I32 = mybir.dt.int32


def _rel_idx():
    coords = np.stack(np.meshgrid(np.arange(G), np.arange(G), indexing="ij")).reshape(2, -1)
    rel = (coords[:, :, None] - coords[:, None, :]).transpose(1, 2, 0)
    rel = rel.copy()
    rel[:, :, 0] += G - 1
    rel[:, :, 1] += G - 1
    rel[:, :, 0] *= 2 * G - 1
    return rel.sum(-1)  # [i, j] in [0, 169)


def _biases_t():
    pb = np.arange(1 - G, G, dtype=np.float32)
    b = np.stack(np.meshgrid(pb, pb, indexing="ij")).reshape(2, -1)  # [2, 169]
    return np.ascontiguousarray(b.astype(np.float32))


_CACHED_NC = None
LAST_RESULTS = None


def _build_nc():
    global _CACHED_NC
    if _CACHED_NC is not None:
        return _CACHED_NC
    nc = bacc.Bacc(None, target_bir_lowering=False)

    qt_d = nc.dram_tensor("qt", [W, H, HD, NTOK], F16, kind="ExternalInput")
    kt_d = nc.dram_tensor("kt", [W, H, HD, NTOK], F16, kind="ExternalInput")
    v_d = nc.dram_tensor("vaug", [W, NTOK, H * 65], F16, kind="ExternalInput")
    id8_d = nc.dram_tensor("ident8", [NTOK, H * NTOK], F16, kind="ExternalInput")
    ridx_d = nc.dram_tensor("relidx", [NTOK, NTOK], I32, kind="ExternalInput")
    bia_d = nc.dram_tensor("biases_t", [2, T], F32, kind="ExternalInput")
    ppw_d = nc.dram_tensor("pos_proj_w", [2, PDIM], F32, kind="ExternalInput")
    ppb_d = nc.dram_tensor("pos_proj_b", [PDIM], F32, kind="ExternalInput")
    mlp_vec = {}
    for nm in ["ln1_g", "ln1_b", "b1", "ln2_g", "ln2_b", "b2", "ln3_g", "ln3_b"]:
        mlp_vec[nm] = nc.dram_tensor(nm, [PDIM], F32, kind="ExternalInput")
    w1_d = nc.dram_tensor("w1", [PDIM, PDIM], F32, kind="ExternalInput")
    w2_d = nc.dram_tensor("w2", [PDIM, PDIM], F32, kind="ExternalInput")
    w3_d = nc.dram_tensor("w3", [PDIM, H], F32, kind="ExternalInput")
    b3_d = nc.dram_tensor("b3", [H], F32, kind="ExternalInput")
    out_d = nc.dram_tensor("out", [W, NTOK, C], F32, kind="ExternalOutput")
    pos_dram = nc.dram_tensor("pos_scratch", [T, H], F32, kind="Internal")

    with tile.TileContext(nc) as tc, ExitStack() as ctx:
        const = ctx.enter_context(tc.tile_pool(name="const", bufs=1))
        mlp = ctx.enter_context(tc.tile_pool(name="mlp", bufs=1))
        mps = ctx.enter_context(tc.tile_pool(name="mps", bufs=1, space="PSUM"))

        # ---------- DynamicPosBias MLP: X^T layout [feat, 169] ----------
        biasesT = mlp.tile([2, T], F32, tag="biasesT")
        nc.sync.dma_start(biasesT[:], bia_d[:])
        ppw = mlp.tile([2, PDIM], F32, tag="ppw")
        nc.sync.dma_start(ppw[:], ppw_d[:])
        vec_sb = {}
        for nm in ["ln1_g", "ln1_b", "b1", "ln2_g", "ln2_b", "b2", "ln3_g", "ln3_b"]:
            t = mlp.tile([PDIM, 1], F32, tag=nm)
            nc.sync.dma_start(t[:], mlp_vec[nm][:])
            vec_sb[nm] = t
        ppb = mlp.tile([PDIM, 1], F32, tag="ppb")
        nc.sync.dma_start(ppb[:], ppb_d[:])
        w1 = mlp.tile([PDIM, PDIM], F32, tag="w1")
        nc.sync.dma_start(w1[:], w1_d[:])
        w2 = mlp.tile([PDIM, PDIM], F32, tag="w2")
        nc.sync.dma_start(w2[:], w2_d[:])
        w3 = mlp.tile([PDIM, H], F32, tag="w3")
        nc.sync.dma_start(w3[:], w3_d[:])
        b3 = mlp.tile([H, 1], F32, tag="b3")
        nc.sync.dma_start(b3[:], b3_d[:])

        eps_t = mlp.tile([1, 1], F32, tag="eps")
        nc.gpsimd.memset(eps_t[:], 1e-5)
        ones_inv = mlp.tile([PDIM, 1], F32, tag="ones_inv")
        nc.gpsimd.memset(ones_inv[:], 1.0 / PDIM)
        ones_row = mlp.tile([1, PDIM], F32, tag="ones_row")
        nc.gpsimd.memset(ones_row[:], 1.0)

        x_ps = mps.tile([PDIM, T], F32, tag="mpsA")
        nc.tensor.matmul(out=x_ps[:], lhsT=ppw[:], rhs=biasesT[:], start=True, stop=True)
        x_sb = mlp.tile([PDIM, T], F32, tag="x_sb")
        nc.vector.tensor_scalar_add(out=x_sb[:], in0=x_ps[:], scalar1=ppb[:])

        layer_params = [
            (vec_sb["ln1_g"], vec_sb["ln1_b"], w1, vec_sb["b1"], PDIM),
            (vec_sb["ln2_g"], vec_sb["ln2_b"], w2, vec_sb["b2"], PDIM),
            (vec_sb["ln3_g"], vec_sb["ln3_b"], w3, b3, H),
        ]
        for li, (g_ap, bln_ap, w_ap, bout_ap, odim) in enumerate(layer_params):
            mu_ps = mps.tile([1, T], F32, tag="mpsA")
            nc.tensor.matmul(out=mu_ps[:], lhsT=ones_inv[:], rhs=x_sb[:], start=True, stop=True)
            mu_sb = mlp.tile([1, T], F32, tag=f"mus{li}")
            nc.vector.tensor_copy(mu_sb[:], mu_ps[:])
            mub_ps = mps.tile([PDIM, T], F32, tag="mpsA")
            nc.tensor.matmul(out=mub_ps[:], lhsT=ones_row[:], rhs=mu_sb[:], start=True, stop=True)
            xc = mlp.tile([PDIM, T], F32, tag=f"xc{li}")
            nc.vector.tensor_tensor(out=xc[:], in0=x_sb[:], in1=mub_ps[:], op=mybir.AluOpType.subtract)
            sq = mlp.tile([PDIM, T], F32, tag=f"sq{li}")
            nc.vector.tensor_tensor(out=sq[:], in0=xc[:], in1=xc[:], op=mybir.AluOpType.mult)
            var_ps = mps.tile([1, T], F32, tag="mpsA")
            nc.tensor.matmul(out=var_ps[:], lhsT=ones_inv[:], rhs=sq[:], start=True, stop=True)
            sd = mlp.tile([1, T], F32, tag=f"sd{li}")
            nc.scalar.activation(sd[:], var_ps[:], mybir.ActivationFunctionType.Sqrt, bias=eps_t[:])
            rstd = mlp.tile([1, T], F32, tag=f"rstd{li}")
            nc.vector.reciprocal(rstd[:], sd[:])
            rstdb_ps = mps.tile([PDIM, T], F32, tag="mpsA")
            nc.tensor.matmul(out=rstdb_ps[:], lhsT=ones_row[:], rhs=rstd[:], start=True, stop=True)
            xh = mlp.tile([PDIM, T], F32, tag=f"xh{li}")
            nc.vector.tensor_tensor(out=xh[:], in0=xc[:], in1=rstdb_ps[:], op=mybir.AluOpType.mult)
            hrelu = mlp.tile([PDIM, T], F32, tag=f"hr{li}")
            nc.scalar.activation(hrelu[:], xh[:], mybir.ActivationFunctionType.Relu,
                                 bias=bln_ap[:], scale=g_ap[:])
            xn_ps = mps.tile([odim, T], F32, tag="mpsA")
            nc.tensor.matmul(out=xn_ps[:], lhsT=w_ap[:], rhs=hrelu[:], start=True, stop=True)
            x_sb = mlp.tile([odim, T], F32, tag=f"xsb{li}")
            nc.vector.tensor_scalar_add(out=x_sb[:], in0=xn_ps[:], scalar1=bout_ap[:])

        # x_sb is now pos^T [8, 169]; push to DRAM as [169, 8] (slow tiny DMA)
        nc.sync.dma_start(pos_dram[:].rearrange("t (h o) -> h t o", o=1), x_sb[:])

        # ---------- gather rpb: 49 row-gathers -> [49, (i,h)] then reorder ----------
        ridx_sb = const.tile([NTOK, NTOK], I32, tag="ridx")
        nc.sync.dma_start(ridx_sb[:], ridx_d[:])
        rpb_tmp = const.tile([NTOK, NTOK * H], F32, tag="rpb_tmp")
        for i in range(NTOK):
            nc.gpsimd.indirect_dma_start(
                out=rpb_tmp[:, H * i : H * i + H],
                out_offset=None,
                in_=pos_dram[:],
                in_offset=bass.IndirectOffsetOnAxis(ap=ridx_sb[:, i : i + 1], axis=0),
            )
        rpb_sb = const.tile([NTOK, H * NTOK], F16, tag="rpb_sb")
        nc.vector.tensor_scalar_mul(
            out=rpb_sb[:].rearrange("p (h i) -> p h i", h=H),
            in0=rpb_tmp[:].rearrange("p (i h) -> p h i", h=H),
            scalar1=8.0,
        )

        # ---------- persistent per-slot QT/KT buffers ----------
        qt_slots = [const.tile([128, H * NTOK], F16, tag=f"qts{s}", name=f"qts{s}") for s in range(NBUF)]
        kt_slots = [const.tile([128, H * NTOK], F16, tag=f"kts{s}", name=f"kts{s}") for s in range(NBUF)]
        for s in range(NBUF):
            nc.sync.dma_start(qt_slots[s][HD : HD + NTOK, :], rpb_sb[:, :])
            nc.sync.dma_start(kt_slots[s][HD : HD + NTOK, :], id8_d[:])

        vpool = ctx.enter_context(tc.tile_pool(name="vpool", bufs=NBUF))
        epool = ctx.enter_context(tc.tile_pool(name="epool", bufs=3))
        opool = ctx.enter_context(tc.tile_pool(name="opool", bufs=3))
        rpool = ctx.enter_context(tc.tile_pool(name="rpool", bufs=3))
        stps = ctx.enter_context(tc.tile_pool(name="stps", bufs=2, space="PSUM"))
        pvps = ctx.enter_context(tc.tile_pool(name="pvps", bufs=2, space="PSUM"))

        for w in range(W):
            s = w % NBUF
            nc.sync.dma_start(
                qt_slots[s][0:HD, :].rearrange("d (h i) -> d h i", h=H),
                qt_d[w].rearrange("h d i -> d h i"),
            )
            nc.sync.dma_start(
                kt_slots[s][0:HD, :].rearrange("d (h i) -> d h i", h=H),
                kt_d[w].rearrange("h d i -> d h i"),
            )
            v_t = vpool.tile([NTOK, H * 65], F16, tag="v")
            nc.sync.dma_start(v_t[:], v_d[w])

            st = stps.tile([NTOK, H * NTOK], F32, tag="st")
            for h in range(H):
                nc.tensor.matmul(
                    out=st[:, NTOK * h : NTOK * (h + 1)],
                    lhsT=kt_slots[s][0 : HD + NTOK, NTOK * h : NTOK * (h + 1)],
                    rhs=qt_slots[s][0 : HD + NTOK, NTOK * h : NTOK * (h + 1)],
                    start=True,
                    stop=True,
                )
            ex = epool.tile([NTOK, H * NTOK], F16, tag="ex")
            nc.scalar.activation(ex[:], st[:], mybir.ActivationFunctionType.Exp, scale=0.125)

            pv0 = pvps.tile([NTOK, 4 * 65], F32, tag="pv0")
            pv1 = pvps.tile([NTOK, 4 * 65], F32, tag="pv1")
            for h in range(H):
                dst = pv0 if h < 4 else pv1
                m = h % 4
                nc.tensor.matmul(
                    out=dst[:, 65 * m : 65 * (m + 1)],
                    lhsT=ex[:, NTOK * h : NTOK * (h + 1)],
                    rhs=v_t[:, 65 * h : 65 * (h + 1)],
                    start=True,
                    stop=True,
                )
            rec = rpool.tile([NTOK, H], F32, tag="rec")
            nc.vector.reciprocal(
                rec[:, 0:4].rearrange("p (h o) -> p h o", o=1),
                pv0[:].rearrange("p (h c) -> p h c", c=65)[:, :, 64:65],
            )
            nc.vector.reciprocal(
                rec[:, 4:8].rearrange("p (h o) -> p h o", o=1),
                pv1[:].rearrange("p (h c) -> p h c", c=65)[:, :, 64:65],
            )
            o_t = opool.tile([NTOK, C], F32, tag="o")
            for half, pv in ((0, pv0), (1, pv1)):
                nc.vector.tensor_tensor(
                    out=o_t[:, 256 * half : 256 * (half + 1)].rearrange(
                        "p (h c) -> p h c", c=HD
                    ),
                    in0=pv[:].rearrange("p (h c) -> p h c", c=65)[:, :, 0:HD],
                    in1=rec[:, 4 * half : 4 * half + 4]
                    .rearrange("p (h o) -> p h o", o=1)
                    .to_broadcast([NTOK, 4, HD]),
                    op=mybir.AluOpType.mult,
                )
            nc.sync.dma_start(out_d[w], o_t[:])

    nc.finalize()
    _CACHED_NC = nc
    return nc


def kernel(q, k, v, pos_proj_w, pos_proj_b, ln1_g, ln1_b, w1, b1,
           ln2_g, ln2_b, w2, b2, ln3_g, ln3_b, w3, b3):
    q = np.ascontiguousarray(np.asarray(q, dtype=np.float32))
    k = np.ascontiguousarray(np.asarray(k, dtype=np.float32))
    v = np.ascontiguousarray(np.asarray(v, dtype=np.float32))

    ident8 = np.tile(np.eye(NTOK, dtype=np.float16), (1, H))
    relidx = np.ascontiguousarray(_rel_idx().T.astype(np.int32))  # [j, i]
    biases_t = _biases_t()

    shared = {
        "ident8": ident8, "relidx": relidx, "biases_t": biases_t,
        "pos_proj_w": np.asarray(pos_proj_w, np.float32),
        "pos_proj_b": np.asarray(pos_proj_b, np.float32),
        "ln1_g": np.asarray(ln1_g, np.float32), "ln1_b": np.asarray(ln1_b, np.float32),
        "w1": np.asarray(w1, np.float32), "b1": np.asarray(b1, np.float32),
        "ln2_g": np.asarray(ln2_g, np.float32), "ln2_b": np.asarray(ln2_b, np.float32),
        "w2": np.asarray(w2, np.float32), "b2": np.asarray(b2, np.float32),
        "ln3_g": np.asarray(ln3_g, np.float32), "ln3_b": np.asarray(ln3_b, np.float32),
        "w3": np.asarray(w3, np.float32), "b3": np.asarray(b3, np.float32),
    }

    ones_col = np.ones((W, NTOK, H, 1), dtype=np.float32)
    in_maps = []
    for c in range(NCORES):
        sl = slice(c * W, (c + 1) * W)
        qt = np.ascontiguousarray(q[sl].reshape(W, NTOK, H, HD).transpose(0, 2, 3, 1).astype(np.float16))
        kt = np.ascontiguousarray(k[sl].reshape(W, NTOK, H, HD).transpose(0, 2, 3, 1).astype(np.float16))
        vaug = np.concatenate(
            [v[sl].reshape(W, NTOK, H, HD), ones_col], axis=3
        ).reshape(W, NTOK, H * 65).astype(np.float16)
        m = dict(shared)
        m.update({"qt": qt, "kt": kt, "vaug": np.ascontiguousarray(vaug)})
        in_maps.append(m)

    nc = _build_nc()
    res = run_bass_kernel_spmd(nc, in_maps, core_ids=list(range(NCORES)))
    global LAST_RESULTS
    LAST_RESULTS = res
    out = np.concatenate([r["out"] for r in res.results], axis=0)
    return out.reshape(B, NTOK, C)



# revision 3
# speedup vs baseline: 3.6489x; 3.6489x over previous
"""Windowed attention (swin-style, 49-token windows, 8 heads) with DynamicPosBias.

Data-parallel over B=2048 windows -> 256 windows/core on 8 cores. The tiny DPB
MLP runs on host (numpy); its output (the 169x8 bias table) is folded into the
QK matmul as 49 identity contraction rows. Windows are processed two-at-a-time
("pairs"): one matmul per (pair, head) computes both windows' 49x49 logits in a
[115,98]x[115,98] product whose cross-window blocks are pushed to -400 via two
extra indicator contraction rows, so exp() underflows them to exactly 0 in fp16.
PV then contracts over all 98 stacked keys against stacked V; softmax
denominators come from N=1 matmuls against a ones vector. 8 windows per
iteration share one input DMA for q+k, one for v, one output DMA.
"""

import numpy as np
from contextlib import ExitStack

import concourse.bass as bass
import concourse.mybir as mybir
import concourse.tile as tile
from concourse import bacc
from concourse.bass_utils import run_bass_kernel_spmd

G = 7
NTOK = 49          # tokens per window
H = 8              # heads
HD = 64            # head dim
C = 512
B = 2048
NCORES = 8
W = B // NCORES    # windows per core (256)
WPI = 8            # windows per iteration
ITERS = W // WPI   # 32
NPAIR = W // 2     # 128 window pairs per core
PDIM = 32          # MLP hidden
NBUF = 3
KROWS = 115        # 64 head dims + 49 identity rows + 2 window-indicator rows
GCOLS = 6272       # (g=4, h=8, t=2, b=2, x=49)
NEG = -400.0       # cross-window mask: exp(0.125 * -400) underflows fp16 to 0
F32 = mybir.dt.float32
F16 = mybir.dt.float16


def _rel_idx():
    coords = np.stack(np.meshgrid(np.arange(G), np.arange(G), indexing="ij")).reshape(2, -1)
    rel = (coords[:, :, None] - coords[:, None, :]).transpose(1, 2, 0).copy()
    rel[:, :, 0] += G - 1
    rel[:, :, 1] += G - 1
    rel[:, :, 0] *= 2 * G - 1
    return rel.sum(-1)  # [query i, key j] in [0, 169)


def _biases():
    pb = np.arange(1 - G, G, dtype=np.float32)
    return np.stack(np.meshgrid(pb, pb, indexing="ij")).reshape(2, -1).T  # [169, 2]


def _ln(x, g, b, eps=1e-5):
    mu = x.mean(-1, keepdims=True)
    var = ((x - mu) ** 2).mean(-1, keepdims=True)
    return (x - mu) / np.sqrt(var + eps) * g + b


def _pos_table(pos_proj_w, pos_proj_b, ln1_g, ln1_b, w1, b1,
               ln2_g, ln2_b, w2, b2, ln3_g, ln3_b, w3, b3):
    x = _biases() @ pos_proj_w + pos_proj_b
    x = np.maximum(_ln(x, ln1_g, ln1_b), 0) @ w1 + b1
    x = np.maximum(_ln(x, ln2_g, ln2_b), 0) @ w2 + b2
    x = np.maximum(_ln(x, ln3_g, ln3_b), 0) @ w3 + b3
    return x.astype(np.float32)  # [169, H]


def _const_rows(pos):
    """Rows 64:115 of each qk slot: identity/rpb + window-indicator rows.

    Col layout (g, h, t, b, x); t=0 is the K half (matmul lhsT), t=1 the Q half
    (matmul rhs). Row 64+j, j<49: K half = I[j,x], Q half = 8*pos[REL_IDX[x,j],h].
    Rows 113/114 add NEG to the (b=0,b'=1) / (b=1,b'=0) cross-window blocks.
    """
    ridx = _rel_idx()  # [query, key]
    rpb8 = 8.0 * pos[ridx]                       # [query x, key j, h]
    c = np.zeros((51, 4, H, 2, 2, NTOK), np.float32)
    eye = np.eye(NTOK, dtype=np.float32)
    c[0:49, :, :, 0, :, :] = eye[:, None, None, None, :]
    c[0:49, :, :, 1, :, :] = rpb8.transpose(1, 2, 0)[:, None, :, None, :]
    c[49, :, :, 0, 0, :] = 1.0   # lhsT row 113: indicator of window b=0
    c[50, :, :, 0, 1, :] = 1.0   # lhsT row 114: indicator of window b=1
    c[49, :, :, 1, 1, :] = NEG   # rhs row 113: -400 on b'=1 columns
    c[50, :, :, 1, 0, :] = NEG   # rhs row 114: -400 on b'=0 columns
    return np.ascontiguousarray(c.reshape(51, GCOLS).astype(np.float16))


_CACHED_NC = None
LAST_RESULTS = None


def _build_nc():
    global _CACHED_NC
    if _CACHED_NC is not None:
        return _CACHED_NC
    nc = bacc.Bacc(None, target_bir_lowering=False)

    qk_d = nc.dram_tensor("qk", [ITERS, HD, GCOLS], F16, kind="ExternalInput")
    cst_d = nc.dram_tensor("qkconst", [51, GCOLS], F16, kind="ExternalInput")
    v_d = nc.dram_tensor("v", [ITERS, 2 * NTOK, 4 * C], F16, kind="ExternalInput")
    out_d = nc.dram_tensor("out", [ITERS, 2 * NTOK, 4 * C], F16, kind="ExternalOutput")

    EXPF = mybir.ActivationFunctionType.Exp
    MULT = mybir.AluOpType.mult

    with tile.TileContext(nc) as tc, ExitStack() as ctx:
        const = ctx.enter_context(tc.tile_pool(name="const", bufs=1))
        qk_slots = [const.tile([KROWS, GCOLS], F16, tag=f"qk{s}", name=f"qk{s}") for s in range(NBUF)]
        v_slots = [const.tile([2 * NTOK, 4 * C], F16, tag=f"v{s}", name=f"v{s}") for s in range(NBUF)]
        for s in range(NBUF):
            nc.sync.dma_start(qk_slots[s][HD:KROWS, :], cst_d[:])
        ones98 = const.tile([2 * NTOK, 1], F16, tag="ones98")
        nc.gpsimd.memset(ones98[:], 1.0)

        stp = ctx.enter_context(tc.tile_pool(name="stp", bufs=2, space="PSUM"))
        pvp = ctx.enter_context(tc.tile_pool(name="pvp", bufs=2, space="PSUM"))
        denp = ctx.enter_context(tc.tile_pool(name="denp", bufs=2, space="PSUM"))
        exq = ctx.enter_context(tc.tile_pool(name="exq", bufs=3))
        recp = ctx.enter_context(tc.tile_pool(name="recp", bufs=2))
        outp = ctx.enter_context(tc.tile_pool(name="outp", bufs=3))

        inflight = {}
        out_holder = {}
        for p in range(NPAIR + 2):
            it, g = divmod(p, 4)
            if p < NPAIR:
                if g == 0:
                    if it == 0:
                        nc.sync.dma_start(qk_slots[0][0:HD, :], qk_d[0])
                        nc.sync.dma_start(v_slots[0][:, :], v_d[0])
                    nxt = it + 1
                    if nxt < ITERS:
                        s = nxt % NBUF
                        nc.sync.dma_start(qk_slots[s][0:HD, :], qk_d[nxt])
                        nc.sync.dma_start(v_slots[s][:, :], v_d[nxt])
                s = it % NBUF
                stA = stp.tile([98, 392], F32, tag="stA")
                stB = stp.tile([98, 392], F32, tag="stB")
                for h in range(H):
                    st = stA if h < 4 else stB
                    j = h % 4
                    base = (g * H + h) * 196
                    nc.tensor.matmul(
                        out=st[:, 98 * j : 98 * j + 98],
                        lhsT=qk_slots[s][0:KROWS, base : base + 98],
                        rhs=qk_slots[s][0:KROWS, base + 98 : base + 196],
                        start=True, stop=True,
                    )
                exA = exq.tile([98, 392], F16, tag="exA")
                exB = exq.tile([98, 392], F16, tag="exB")
                nc.scalar.activation(exA[:], stA[:], EXPF, scale=0.125)
                nc.scalar.activation(exB[:], stB[:], EXPF, scale=0.125)
                inflight[p] = (exA, exB, s, g, it)
            if p >= 2:
                exA, exB, s2, g2, it2 = inflight.pop(p - 2)
                pv = pvp.tile([98, 512], F32, tag="pv")
                den = denp.tile([98, 8], F32, tag="den")
                for h in range(H):
                    ex = exA if h < 4 else exB
                    lhs = ex[:, 98 * (h % 4) : 98 * (h % 4) + 98]
                    nc.tensor.matmul(
                        out=pv[:, HD * h : HD * h + HD],
                        lhsT=lhs,
                        rhs=v_slots[s2][:, C * g2 + HD * h : C * g2 + HD * h + HD],
                        start=True, stop=True,
                    )
                    nc.tensor.matmul(
                        out=den[:, h : h + 1], lhsT=lhs, rhs=ones98[:],
                        start=True, stop=True,
                    )
                rec = recp.tile([98, 8], F32, tag="rec")
                nc.vector.reciprocal(rec[:], den[:])
                if g2 == 0:
                    out_t = outp.tile([98, 4 * C], F16, tag="out", name=f"out{it2}")
                    out_holder[it2] = out_t
                out_t = out_holder[it2]
                nc.vector.tensor_tensor(
                    out=out_t[:, C * g2 : C * g2 + C].rearrange("p (h c) -> p h c", c=HD),
                    in0=pv[:].rearrange("p (h c) -> p h c", c=HD),
                    in1=rec[:].rearrange("p (h o) -> p h o", o=1).to_broadcast([98, H, HD]),
                    op=MULT,
                )
                if g2 == 3:
                    nc.gpsimd.dma_start(out_d[it2], out_holder.pop(it2)[:])

    nc.finalize()
    _CACHED_NC = nc
    return nc


def kernel(q, k, v, pos_proj_w, pos_proj_b, ln1_g, ln1_b, w1, b1,
           ln2_g, ln2_b, w2, b2, ln3_g, ln3_b, w3, b3):
    q = np.asarray(q, dtype=np.float32)
    k = np.asarray(k, dtype=np.float32)
    v = np.asarray(v, dtype=np.float32)

    pos = _pos_table(
        np.asarray(pos_proj_w, np.float32), np.asarray(pos_proj_b, np.float32),
        np.asarray(ln1_g, np.float32), np.asarray(ln1_b, np.float32),
        np.asarray(w1, np.float32), np.asarray(b1, np.float32),
        np.asarray(ln2_g, np.float32), np.asarray(ln2_b, np.float32),
        np.asarray(w2, np.float32), np.asarray(b2, np.float32),
        np.asarray(ln3_g, np.float32), np.asarray(ln3_b, np.float32),
        np.asarray(w3, np.float32), np.asarray(b3, np.float32),
    )
    cst = _const_rows(pos)

    in_maps = []
    for c in range(NCORES):
        sl = slice(c * W, (c + 1) * W)
        # qk: [iters, 64, (g, h, t, b, x)] with t=0 K-data, t=1 Q-data
        qc = q[sl].reshape(ITERS, 4, 2, NTOK, H, HD).transpose(0, 5, 1, 4, 2, 3)
        kc = k[sl].reshape(ITERS, 4, 2, NTOK, H, HD).transpose(0, 5, 1, 4, 2, 3)
        qk = np.stack([kc, qc], axis=4).reshape(ITERS, HD, GCOLS).astype(np.float16)
        # v: [iters, (b, k), (g, h, c)]
        vc = v[sl].reshape(ITERS, 4, 2, NTOK, C).transpose(0, 2, 3, 1, 4)
        vc = vc.reshape(ITERS, 2 * NTOK, 4 * C).astype(np.float16)
        in_maps.append({
            "qk": np.ascontiguousarray(qk),
            "v": np.ascontiguousarray(vc),
            "qkconst": cst,
        })

    nc = _build_nc()
    res = run_bass_kernel_spmd(nc, in_maps, core_ids=list(range(NCORES)))
    global LAST_RESULTS
    LAST_RESULTS = res
    # out: [iters, (b, q), (g, c)] -> [W, 49, 512]
    parts = []
    for r in res.results:
        o = r["out"].reshape(ITERS, 2, NTOK, 4, C).transpose(0, 3, 1, 2, 4)
        parts.append(o.reshape(W, NTOK, C))
    return np.concatenate(parts, axis=0).astype(np.float32)


# revision 8
# speedup vs baseline: 4.2282x; 1.1588x over previous
"""Windowed attention (swin-style, 49-token windows, 8 heads) with DynamicPosBias.

Data-parallel over B=2048 windows -> 256 windows/core on 8 cores. The tiny DPB
MLP runs on host (numpy); its output (the 169x8 bias table) is folded into the
QK matmul as 49 identity contraction rows. Windows are processed two-at-a-time
("pairs"): one matmul per (pair, head) computes both windows' 49x49 logits in a
[115,98]x[115,98] product whose cross-window blocks are pushed to -400 via two
extra indicator contraction rows, so exp() underflows them to exactly 0 in fp16.
PV then contracts over all 98 stacked keys against stacked V; softmax
denominators come from N=1 matmuls against a ones vector. 8 windows per
iteration share one input DMA for q+k, one for v, one output DMA.
"""

import numpy as np
from contextlib import ExitStack

import concourse.bass as bass
import concourse.mybir as mybir
import concourse.tile as tile
from concourse import bacc
from concourse.bass_utils import run_bass_kernel_spmd

G = 7
NTOK = 49          # tokens per window
H = 8              # heads
HD = 64            # head dim
C = 512
B = 2048
NCORES = 8
W = B // NCORES    # windows per core (256)
WPI = 8            # windows per iteration
ITERS = W // WPI   # 32
NPAIR = W // 2     # 128 window pairs per core
PDIM = 32          # MLP hidden
NBUF = 3
KROWS = 115        # 64 head dims + 49 identity rows + 2 window-indicator rows
GCOLS = 6272       # (g=4, h=8, t=2, b=2, x=49)
NEG = -400.0       # cross-window mask: exp(0.125 * -400) underflows fp16 to 0
F32 = mybir.dt.float32
F16 = mybir.dt.float16


def _rel_idx():
    coords = np.stack(np.meshgrid(np.arange(G), np.arange(G), indexing="ij")).reshape(2, -1)
    rel = (coords[:, :, None] - coords[:, None, :]).transpose(1, 2, 0).copy()
    rel[:, :, 0] += G - 1
    rel[:, :, 1] += G - 1
    rel[:, :, 0] *= 2 * G - 1
    return rel.sum(-1)  # [query i, key j] in [0, 169)


def _biases():
    pb = np.arange(1 - G, G, dtype=np.float32)
    return np.stack(np.meshgrid(pb, pb, indexing="ij")).reshape(2, -1).T  # [169, 2]


def _ln(x, g, b, eps=1e-5):
    mu = x.mean(-1, keepdims=True)
    var = ((x - mu) ** 2).mean(-1, keepdims=True)
    return (x - mu) / np.sqrt(var + eps) * g + b


def _pos_table(pos_proj_w, pos_proj_b, ln1_g, ln1_b, w1, b1,
               ln2_g, ln2_b, w2, b2, ln3_g, ln3_b, w3, b3):
    x = _biases() @ pos_proj_w + pos_proj_b
    x = np.maximum(_ln(x, ln1_g, ln1_b), 0) @ w1 + b1
    x = np.maximum(_ln(x, ln2_g, ln2_b), 0) @ w2 + b2
    x = np.maximum(_ln(x, ln3_g, ln3_b), 0) @ w3 + b3
    return x.astype(np.float32)  # [169, H]


def _const_rows(pos):
    """Rows 64:115 of each qk slot: identity/rpb + window-indicator rows.

    Col layout (g, h, t, b, x); t=0 is the K half (matmul lhsT), t=1 the Q half
    (matmul rhs). Row 64+j, j<49: K half = I[j,x], Q half = 8*pos[REL_IDX[x,j],h].
    Rows 113/114 add NEG to the (b=0,b'=1) / (b=1,b'=0) cross-window blocks.
    """
    ridx = _rel_idx()  # [query, key]
    rpb8 = 8.0 * pos[ridx]                       # [query x, key j, h]
    c = np.zeros((51, 4, H, 2, 2, NTOK), np.float32)
    eye = np.eye(NTOK, dtype=np.float32)
    c[0:49, :, :, 0, :, :] = eye[:, None, None, None, :]
    c[0:49, :, :, 1, :, :] = rpb8.transpose(1, 2, 0)[:, None, :, None, :]
    c[49, :, :, 0, 0, :] = 1.0   # lhsT row 113: indicator of window b=0
    c[50, :, :, 0, 1, :] = 1.0   # lhsT row 114: indicator of window b=1
    c[49, :, :, 1, 1, :] = NEG   # rhs row 113: -400 on b'=1 columns
    c[50, :, :, 1, 0, :] = NEG   # rhs row 114: -400 on b'=0 columns
    return np.ascontiguousarray(c.reshape(51, GCOLS).astype(np.float16))


_CACHED_NC = None
LAST_RESULTS = None


def _build_nc():
    global _CACHED_NC
    if _CACHED_NC is not None:
        return _CACHED_NC
    nc = bacc.Bacc(None, target_bir_lowering=False)

    VW = 4 * H * 65  # v slot cols: (g, h, c65) with fused ones column
    qk_d = nc.dram_tensor("qk", [ITERS, HD, GCOLS], F16, kind="ExternalInput")
    cst_d = nc.dram_tensor("qkconst", [51, GCOLS], F16, kind="ExternalInput")
    v_d = nc.dram_tensor("v", [ITERS, 2 * NTOK, VW], F16, kind="ExternalInput")
    out_d = nc.dram_tensor("out", [ITERS, 2 * NTOK, 4 * C], F16, kind="ExternalOutput")

    EXPF = mybir.ActivationFunctionType.Exp
    MULT = mybir.AluOpType.mult

    with tile.TileContext(nc) as tc, ExitStack() as ctx:
        const = ctx.enter_context(tc.tile_pool(name="const", bufs=1))
        qk_slots = [const.tile([KROWS, GCOLS], F16, tag=f"qk{s}", name=f"qk{s}") for s in range(NBUF)]
        v_slots = [const.tile([2 * NTOK, VW], F16, tag=f"v{s}", name=f"v{s}") for s in range(NBUF)]
        for s in range(NBUF):
            nc.sync.dma_start(qk_slots[s][HD:KROWS, :], cst_d[:])

        stp = ctx.enter_context(tc.tile_pool(name="stp", bufs=2, space="PSUM"))
        pvp = ctx.enter_context(tc.tile_pool(name="pvp", bufs=2, space="PSUM"))
        exq = ctx.enter_context(tc.tile_pool(name="exq", bufs=3))
        recp = ctx.enter_context(tc.tile_pool(name="recp", bufs=2))
        outp = ctx.enter_context(tc.tile_pool(name="outp", bufs=3))

        inflight = {}
        out_holder = {}
        for p in range(NPAIR + 2):
            it, g = divmod(p, 4)
            if p < NPAIR:
                if g == 0:
                    if it == 0:
                        nc.sync.dma_start(qk_slots[0][0:HD, :], qk_d[0])
                        nc.sync.dma_start(v_slots[0][:, :], v_d[0])
                    nxt = it + 1
                    if nxt < ITERS:
                        s = nxt % NBUF
                        nc.sync.dma_start(qk_slots[s][0:HD, :], qk_d[nxt])
                        nc.sync.dma_start(v_slots[s][:, :], v_d[nxt])
                s = it % NBUF
                stA = stp.tile([98, 392], F32, tag="stA")
                stB = stp.tile([98, 392], F32, tag="stB")
                for h in range(H):
                    st = stA if h < 4 else stB
                    j = h % 4
                    base = (g * H + h) * 196
                    nc.tensor.matmul(
                        out=st[:, 98 * j : 98 * j + 98],
                        lhsT=qk_slots[s][0:KROWS, base : base + 98],
                        rhs=qk_slots[s][0:KROWS, base + 98 : base + 196],
                        start=True, stop=True,
                    )
                exA = exq.tile([98, 392], F16, tag="exA")
                exB = exq.tile([98, 392], F16, tag="exB")
                nc.scalar.activation(exA[:], stA[:], EXPF, scale=0.125)
                nc.scalar.activation(exB[:], stB[:], EXPF, scale=0.125)
                inflight[p] = (exA, exB, s, g, it)
            if p >= 2:
                exA, exB, s2, g2, it2 = inflight.pop(p - 2)
                pvA = pvp.tile([98, 260], F32, tag="pvA")
                pvB = pvp.tile([98, 260], F32, tag="pvB")
                for h in range(H):
                    ex = exA if h < 4 else exB
                    pv = pvA if h < 4 else pvB
                    j = h % 4
                    nc.tensor.matmul(
                        out=pv[:, 65 * j : 65 * j + 65],
                        lhsT=ex[:, 98 * j : 98 * j + 98],
                        rhs=v_slots[s2][:, 520 * g2 + 65 * h : 520 * g2 + 65 * h + 65],
                        start=True, stop=True,
                    )
                if g2 == 0:
                    out_t = outp.tile([98, 4 * C], F16, tag="out", name=f"out{it2}")
                    out_holder[it2] = out_t
                out_t = out_holder[it2]
                for t, pv in ((0, pvA), (1, pvB)):
                    rec = recp.tile([98, 4], F32, tag=f"rec{t}", name=f"rec{t}")
                    nc.vector.reciprocal(
                        rec[:].rearrange("p (h o) -> p h o", o=1),
                        pv[:].rearrange("p (h c) -> p h c", c=65)[:, :, 64:65],
                    )
                    nc.vector.tensor_tensor(
                        out=out_t[:, C * g2 + 256 * t : C * g2 + 256 * t + 256]
                            .rearrange("p (h c) -> p h c", c=HD),
                        in0=pv[:].rearrange("p (h c) -> p h c", c=65)[:, :, 0:HD],
                        in1=rec[:].rearrange("p (h o) -> p h o", o=1).to_broadcast([98, 4, HD]),
                        op=MULT,
                    )
                if g2 == 3:
                    nc.gpsimd.dma_start(out_d[it2], out_holder.pop(it2)[:])

    nc.finalize()
    _CACHED_NC = nc
    return nc


def kernel(q, k, v, pos_proj_w, pos_proj_b, ln1_g, ln1_b, w1, b1,
           ln2_g, ln2_b, w2, b2, ln3_g, ln3_b, w3, b3):
    q = np.asarray(q, dtype=np.float32)
    k = np.asarray(k, dtype=np.float32)
    v = np.asarray(v, dtype=np.float32)

    pos = _pos_table(
        np.asarray(pos_proj_w, np.float32), np.asarray(pos_proj_b, np.float32),
        np.asarray(ln1_g, np.float32), np.asarray(ln1_b, np.float32),
        np.asarray(w1, np.float32), np.asarray(b1, np.float32),
        np.asarray(ln2_g, np.float32), np.asarray(ln2_b, np.float32),
        np.asarray(w2, np.float32), np.asarray(b2, np.float32),
        np.asarray(ln3_g, np.float32), np.asarray(ln3_b, np.float32),
        np.asarray(w3, np.float32), np.asarray(b3, np.float32),
    )
    cst = _const_rows(pos)

    in_maps = []
    for c in range(NCORES):
        sl = slice(c * W, (c + 1) * W)
        # qk: [iters, 64, (g, h, t, b, x)] with t=0 K-data, t=1 Q-data
        qc = q[sl].reshape(ITERS, 4, 2, NTOK, H, HD).transpose(0, 5, 1, 4, 2, 3)
        kc = k[sl].reshape(ITERS, 4, 2, NTOK, H, HD).transpose(0, 5, 1, 4, 2, 3)
        qk = np.stack([kc, qc], axis=4).reshape(ITERS, HD, GCOLS).astype(np.float16)
        # v: [iters, (b, k), (g, h, c65)] with ones column per head (denominator)
        vc = v[sl].reshape(W, NTOK, H, HD)
        vc = np.concatenate([vc, np.ones((W, NTOK, H, 1), np.float32)], axis=3)
        vc = vc.reshape(ITERS, 4, 2, NTOK, H * 65).transpose(0, 2, 3, 1, 4)
        vc = vc.reshape(ITERS, 2 * NTOK, 4 * H * 65).astype(np.float16)
        in_maps.append({
            "qk": np.ascontiguousarray(qk),
            "v": np.ascontiguousarray(vc),
            "qkconst": cst,
        })

    nc = _build_nc()
    res = run_bass_kernel_spmd(nc, in_maps, core_ids=list(range(NCORES)))
    global LAST_RESULTS
    LAST_RESULTS = res
    # out: [iters, (b, q), (g, c)] -> [W, 49, 512]
    parts = []
    for r in res.results:
        o = r["out"].reshape(ITERS, 2, NTOK, 4, C).transpose(0, 3, 1, 2, 4)
        parts.append(o.reshape(W, NTOK, C))
    return np.concatenate(parts, axis=0).astype(np.float32)
